# revision 5
# baseline (speedup 1.0000x reference)
"""Quadrilinear (4D separable) interpolation kernel for Trainium2.

Problem: input [1, 16, 5, 5, 128, 128] f32 -> output [1, 16, 8, 8, 256, 256]
(align_corners=True linear interp along X:5->8, Y:5->8, U:128->256, V:128->256).

Strategy (channel-parallel, no communication):
  - 16 channels sharded 2-per-core across 8 NeuronCores.
  - Interp along each axis is a linear map. U/V maps are a 128x256 matrix L
    (2 nonzeros per column); X/Y maps are 1-2 scalar-weighted taps.
  - Per core, two matmul stages on the tensor engine (data = stationary lhsT,
    scaled copies of L = moving rhs), accumulating X/Y taps in PSUM:
      stage 1: B[vi, uo](c,xo,yi) = sum_xi wx(xo,xi) * (R[c,xi,yi].T @ L)
      stage 2: out[uo_chunk, vo](c,xo,yo) = sum_yi wy(yo,yi) * (B_chunk.T @ L)
    Both contractions are over the 128-partition axis; no transposes needed.
  - PSUM -> SBUF copies alternate between the vector and scalar engines;
    output staged per (c,xo) as [128, 8, 2, 256] and DMA'd as one 2 MiB burst.
"""

import numpy as np

N_CORES = 8
C_TOTAL = 16
C_PER_CORE = C_TOTAL // N_CORES
XI, YI = 5, 5
XO, YO = 8, 8
UI, VI = 128, 128
UO, VO = 256, 256
P = 128

_CACHE = {}
LAST_RESULTS = None  # BassKernelResults from the most recent run (for profiling)
RUN_KWARGS = {}  # extra kwargs for run_bass_kernel_spmd (test harness sets trace=True)


def _lerp_meta(in_size, out_size):
    """Mirror reference.py's float32 coordinate math (align_corners=True)."""
    scale = (in_size - 1) / (out_size - 1)
    coords = np.arange(out_size, dtype=np.float32) * np.float32(scale)
    i0 = np.clip(np.floor(coords).astype(np.int32), 0, in_size - 1)
    i1 = np.minimum(i0 + 1, in_size - 1)
    w = (coords - i0.astype(np.float32)).astype(np.float32)
    return i0, i1, w


def _taps_small():
    """Per-xo list of (input_index, float32_weight) for the 5->8 axes."""
    i0, i1, w = _lerp_meta(XI, XO)
    taps = []
    for j in range(XO):
        if i0[j] == i1[j]:
            taps.append([(int(i0[j]), np.float32((np.float32(1.0) - w[j]) + w[j]))])
        elif w[j] == 0.0:
            taps.append([(int(i0[j]), np.float32(1.0))])
        else:
            taps.append(
                [
                    (int(i0[j]), np.float32(np.float32(1.0) - w[j])),
                    (int(i1[j]), np.float32(w[j])),
                ]
            )
    return taps


def _l_matrix():
    """L[ui, uo]: weight of input row ui on output row uo for the 128->256 axes."""
    i0, i1, w = _lerp_meta(UI, UO)
    L = np.zeros((UI, UO), dtype=np.float32)
    for j in range(UO):
        L[i0[j], j] += np.float32(1.0) - w[j]
        L[i1[j], j] += w[j]
    return L


def _build_constants():
    taps = _taps_small()
    L = _l_matrix()
    # Distinct tap weights (by exact f32 bits) -> scaled copies of L.
    wvals = []
    windex = {}
    for tlist in taps:
        for _, wv in tlist:
            key = np.float32(wv).tobytes()
            if key not in windex:
                windex[key] = len(wvals)
                wvals.append(np.float32(wv))
    # Wcat[ui, nw, uo] = wvals[nw] * L[ui, uo]
    Wcat = np.stack([np.float32(wv) * L for wv in wvals], axis=0)  # [nw, 128, 256]
    Wcat = np.ascontiguousarray(Wcat.transpose(1, 0, 2))  # [128, nw, 256]
    taps_idx = [[(i, windex[np.float32(wv).tobytes()]) for (i, wv) in tl] for tl in taps]
    return taps_idx, Wcat


def _build_bass(nw):
    import concourse.tile as tile
    from concourse import bacc, mybir

    taps_idx, _ = _build_constants()
    f32 = mybir.dt.float32

    nc = bacc.Bacc(None, target_bir_lowering=False)
    x_ext = nc.declare_dram_parameter(
        "x", [P, C_PER_CORE, XI, YI, VI], f32, isOutput=False
    )
    w_ext = nc.declare_dram_parameter("w", [P, nw, UO], f32, isOutput=False)
    out_ext = nc.declare_dram_parameter(
        "out", [C_PER_CORE, XO, YO, 2, P, VO], f32, isOutput=True
    )

    copy_ctr = [0]

    with tile.TileContext(nc) as tc:
        with (
            tc.tile_pool(name="const", bufs=1) as constp,
            tc.tile_pool(name="bpool", bufs=1) as bpool,
            tc.tile_pool(name="outp", bufs=3) as outp,
            tc.tile_pool(name="psum", bufs=8, space="PSUM") as psum,
        ):
            R = constp.tile([P, C_PER_CORE, XI, YI, VI], f32)
            W = constp.tile([P, nw, UO], f32)
            nc.sync.dma_start(R[:], x_ext[:])
            nc.sync.dma_start(W[:], w_ext[:])

            B = bpool.tile([P, C_PER_CORE, XO, YI, UO], f32)

            def copy_out(dst, src):
                if copy_ctr[0] % 2 == 0:
                    nc.vector.tensor_copy(dst, src)
                else:
                    nc.scalar.activation(
                        dst, src, mybir.ActivationFunctionType.Copy
                    )
                copy_ctr[0] += 1

            # Stage 1: U-interp with X taps folded into PSUM accumulation.
            for c in range(C_PER_CORE):
                for xo in range(XO):
                    for yi in range(YI):
                        ps = psum.tile([P, UO], f32, tag="ps")
                        tl = taps_idx[xo]
                        for k, (xi, widx) in enumerate(tl):
                            nc.tensor.matmul(
                                ps[:],
                                R[:, c, xi, yi, :],
                                W[:, widx, :],
                                start=(k == 0),
                                stop=(k == len(tl) - 1),
                            )
                        copy_out(B[:, c, xo, yi, :], ps[:])

            # Stage 2: V-interp with Y taps folded into PSUM accumulation.
            for c in range(C_PER_CORE):
                for xo in range(XO):
                    ob = outp.tile([P, YO, 2, VO], f32, tag="ob")
                    for yo in range(YO):
                        tl = taps_idx[yo]
                        for uoc in range(2):
                            ps2 = psum.tile([P, VO], f32, tag="ps")
                            for k, (yi, widx) in enumerate(tl):
                                nc.tensor.matmul(
                                    ps2[:],
                                    B[:, c, xo, yi, uoc * P : (uoc + 1) * P],
                                    W[:, widx, :],
                                    start=(k == 0),
                                    stop=(k == len(tl) - 1),
                                )
                            copy_out(ob[:, yo, uoc, :], ps2[:])
                    # out_ext[c, xo]: [yo, uoc, uo, vo]; iterate as [uo, yo, uoc, vo].
                    nc.sync.dma_start(
                        out_ext[c, xo].transpose([2, 0, 1, 3]), ob[:]
                    )
    nc.finalize()
    return nc


def kernel(input: np.ndarray) -> np.ndarray:
    global LAST_RESULTS
    from concourse.bass_utils import run_bass_kernel_spmd

    assert input.shape == (1, C_TOTAL, XI, YI, UI, VI), input.shape
    x = np.asarray(input, dtype=np.float32)

    if "nc" not in _CACHE:
        _, Wcat = _build_constants()
        _CACHE["Wcat"] = Wcat
        _CACHE["nc"] = _build_bass(Wcat.shape[1])
    Wcat = _CACHE["Wcat"]
    nc = _CACHE["nc"]

    in_maps = []
    for k in range(N_CORES):
        slab = x[0, k * C_PER_CORE : (k + 1) * C_PER_CORE]  # [2,5,5,128,128]
        slab_t = np.ascontiguousarray(slab.transpose(3, 0, 1, 2, 4))  # [ui,c,xi,yi,vi]
        in_maps.append({"x": slab_t, "w": Wcat})

    res = run_bass_kernel_spmd(nc, in_maps, list(range(N_CORES)), **RUN_KWARGS)
    LAST_RESULTS = res

    outs = [
        np.asarray(res.results[k]["out"]).reshape(C_PER_CORE, XO, YO, UO, VO)
        for k in range(N_CORES)
    ]
    return np.concatenate(outs, axis=0)[None].astype(np.float32)
